# revision 14
# baseline (speedup 1.0000x reference)
"""Trainium2 Bass kernel for a BiAttention module (data-parallel over batch).

Computes, per batch item b:
  i_feat = attn(q=i_b, kv=q_b, mask=q_mask)        [256, 512]
  i_flat, i_weight = att_flat(i_feat, i_mask, lf_*)
  l_feat = attn(q=q_b, kv=i_b, mask=i_mask)        [64, 512]
  l_flat, _ = att_flat(l_feat, q_mask, if_*)
  out = (l_flat + i_flat) @ fh_w + fh_b            [512]
Returns (out [B,512], i_weight [B,256]).

Strategy: 8-way data parallel over batch (32 items/core), weights replicated.
Single score matrix S = q_b @ i_b^T serves both attentions (S and S^T computed
on PE from DMA-transposed bf16 operand layouts). Softmax uses exp without max
subtraction (scores are O(1); masked lanes get -1e9 bias -> exp underflows to
exactly 0). Softmax-Z normalizations are deferred through the relu MLP via
positive homogeneity and applied as per-partition scalars.
"""

import numpy as np
import ml_dtypes
from contextlib import ExitStack

import concourse.bass as bass
import concourse.bacc as bacc
import concourse.mybir as mybir
import concourse.tile as tile
from concourse.bass_utils import run_bass_kernel_spmd

B, ILEN, QLEN, H = 256, 256, 64, 512
MID, FLAT, OUT = 512, 1024, 512
NCORES = 8
F32 = mybir.dt.float32
BF16 = mybir.dt.bfloat16
SCALE = float(1.0 / np.sqrt(H))
AF = mybir.ActivationFunctionType


def build(C=B // NCORES, NB=4, include_b1=False):
    nc = bacc.Bacc()

    # ---- DRAM parameters (inputs) ----
    i_bf = nc.declare_dram_parameter("i_bf", [C, ILEN, H], BF16, isOutput=False)
    q_bf = nc.declare_dram_parameter("q_bf", [C, QLEN, H], BF16, isOutput=False)
    iT_d = nc.declare_dram_parameter("iT_d", [C, H, ILEN], BF16, isOutput=False)
    qT_d = nc.declare_dram_parameter("qT_d", [C, H, QLEN], BF16, isOutput=False)
    mbq = nc.declare_dram_parameter("mbq", [C, QLEN], F32, isOutput=False)
    mbi = nc.declare_dram_parameter("mbi", [C, ILEN], F32, isOutput=False)
    abi = nc.declare_dram_parameter("abi", [C, ILEN], F32, isOutput=False)
    abq = nc.declare_dram_parameter("abq", [C, QLEN], F32, isOutput=False)
    w1lf = nc.declare_dram_parameter("w1lf", [H, MID], BF16, isOutput=False)
    w1if = nc.declare_dram_parameter("w1if", [H, MID], BF16, isOutput=False)
    w2lf = nc.declare_dram_parameter("w2lf", [MID], BF16, isOutput=False)
    w2if = nc.declare_dram_parameter("w2if", [MID], BF16, isOutput=False)
    wmlf = nc.declare_dram_parameter("wmlf", [H, FLAT], BF16, isOutput=False)
    wmif = nc.declare_dram_parameter("wmif", [H, FLAT], BF16, isOutput=False)
    fhw = nc.declare_dram_parameter("fhw", [FLAT, OUT], BF16, isOutput=False)
    fhb = nc.declare_dram_parameter("fhb", [OUT], F32, isOutput=False)
    b1lf = nc.declare_dram_parameter("b1lf", [MID], BF16, isOutput=False)
    b1if = nc.declare_dram_parameter("b1if", [MID], BF16, isOutput=False)
    # ---- outputs ----
    outp = nc.declare_dram_parameter("outp", [C, OUT], F32, isOutput=True)
    iwp = nc.declare_dram_parameter("iwp", [C, ILEN], F32, isOutput=True)

    with ExitStack() as ctx:
        tc = ctx.enter_context(tile.TileContext(nc))
        const = ctx.enter_context(tc.tile_pool(name="const", bufs=1))
        iop = ctx.enter_context(tc.tile_pool(name="iop", bufs=3))
        wk = ctx.enter_context(tc.tile_pool(name="wk", bufs=3))
        pp = ctx.enter_context(tc.tile_pool(name="pp", bufs=1, space="PSUM"))

        # ---- constants / weights resident in SBUF ----
        w1lf_t = const.tile([128, 4, MID], BF16, tag="w1lf")
        nc.sync.dma_start(out=w1lf_t[:], in_=w1lf.rearrange("(k p) m -> p k m", p=128))
        w1if_t = const.tile([128, 4, MID], BF16, tag="w1if")
        nc.sync.dma_start(out=w1if_t[:], in_=w1if.rearrange("(k p) m -> p k m", p=128))
        w2lf_t = const.tile([128, 4], BF16, tag="w2lf")
        nc.sync.dma_start(out=w2lf_t[:], in_=w2lf.rearrange("(k p) -> p k", p=128))
        w2if_t = const.tile([128, 4], BF16, tag="w2if")
        nc.sync.dma_start(out=w2if_t[:], in_=w2if.rearrange("(k p) -> p k", p=128))
        wmlf_t = const.tile([128, 4, FLAT], BF16, tag="wmlf")
        nc.sync.dma_start(out=wmlf_t[:], in_=wmlf.rearrange("(k p) m -> p k m", p=128))
        wmif_t = const.tile([128, 4, FLAT], BF16, tag="wmif")
        nc.sync.dma_start(out=wmif_t[:], in_=wmif.rearrange("(k p) m -> p k m", p=128))
        fh_t = const.tile([128, 8, OUT], BF16, tag="fh")
        nc.sync.dma_start(out=fh_t[:], in_=fhw.rearrange("(k p) m -> p k m", p=128))
        fhb_t = const.tile([128, 4], F32, tag="fhb")
        nc.sync.dma_start(out=fhb_t[:], in_=fhb.rearrange("(j p) -> p j", p=128))

        mbq_t = const.tile([QLEN, C], F32, tag="mbq")
        nc.sync.dma_start(out=mbq_t[:], in_=mbq.rearrange("b a -> a b"))
        mbi_t = const.tile([128, C, 2], F32, tag="mbi")
        nc.sync.dma_start(out=mbi_t[:], in_=mbi.rearrange("b (t p) -> p (b t)", p=128))
        abi_t = const.tile([128, C, 2], F32, tag="abi")
        nc.sync.dma_start(out=abi_t[:], in_=abi.rearrange("b (t p) -> p (b t)", p=128))
        abq_t = const.tile([QLEN, C], F32, tag="abq")
        nc.sync.dma_start(out=abq_t[:], in_=abq.rearrange("b a -> a b"))

        ones_bf = const.tile([128, 1], BF16, tag="ones_bf")
        nc.vector.memset(ones_bf[:], 1.0)
        ones_f = const.tile([128, 1], F32, tag="ones_f")
        nc.vector.memset(ones_f[:], 1.0)
        onesr_f = const.tile([1, 128], F32, tag="onesr_f")
        nc.vector.memset(onesr_f[:], 1.0)

        if include_b1:
            b1lf_t = const.tile([1, MID], BF16, tag="b1lf")
            nc.sync.dma_start(out=b1lf_t[:], in_=b1lf.rearrange("(o m) -> o m", o=1))
            b1if_t = const.tile([1, MID], BF16, tag="b1if")
            nc.sync.dma_start(out=b1if_t[:], in_=b1if.rearrange("(o m) -> o m", o=1))

        # persistent accumulators / output staging
        att_all = const.tile([128, C, 2], F32, tag="att_all")
        pooled_sb = const.tile([128, 4, 2, C], BF16, tag="pooled_sb")
        flatT_sb = const.tile([128, 8, C], BF16, tag="flatT_sb")
        outT_sb = const.tile([128, C, 4], F32, tag="outT_sb")
        pooled_ps = pp.tile([128, 4, 2, C], F32, tag="pooled")

        n_groups = C // NB
        for g in range(n_groups):
            bsl = slice(g * NB, (g + 1) * NB)
            # --- transposed layouts (h on partitions), pre-transposed on host ---
            iT_t = iop.tile([128, NB, 4, ILEN], BF16, tag="iT")
            nc.scalar.dma_start(
                out=iT_t[:], in_=iT_d[bsl].rearrange("b (k p) c -> p (b k) c", p=128)
            )
            qT_t = iop.tile([128, NB, 4, QLEN], BF16, tag="qT")
            nc.scalar.dma_start(
                out=qT_t[:], in_=qT_d[bsl].rearrange("b (k p) c -> p (b k) c", p=128)
            )
            # --- natural loads (seq on partitions) ---
            inat_t = iop.tile([128, NB * 2, H], BF16, tag="inat")
            nc.scalar.dma_start(
                out=inat_t[:], in_=i_bf[bsl].rearrange("b (t p) h -> p (b t) h", p=128)
            )
            qn_t = iop.tile([QLEN, NB, H], BF16, tag="qn")
            nc.scalar.dma_start(out=qn_t[:], in_=q_bf[bsl].rearrange("b r h -> r b h"))

            for bb in range(NB):
                b = g * NB + bb
                # ============ scores: S [64,256] and S^T [256,64] ============
                scores_t = pp.tile([128, 512], F32, tag="scores", bufs=2)
                St = scores_t[:, 0:128].rearrange("p (t a) -> p t a", a=64)
                S = scores_t[0:QLEN, 128 : 128 + ILEN]
                for k in range(4):
                    nc.tensor.matmul(
                        S,
                        qT_t[:, bb, k, :],
                        iT_t[:, bb, k, :],
                        start=(k == 0),
                        stop=(k == 3),
                    )
                for ct in range(2):
                    for k in range(4):
                        nc.tensor.matmul(
                            St[:, ct, :],
                            iT_t[:, bb, k, 128 * ct : 128 * (ct + 1)],
                            qT_t[:, bb, k, :],
                            start=(k == 0),
                            stop=(k == 3),
                        )
                # ============ masked exps ============
                E_t = wk.tile([QLEN, ILEN], BF16, tag="E")
                nc.scalar.activation(
                    E_t[:], S, AF.Exp, bias=mbq_t[:, b : b + 1], scale=SCALE
                )
                ElT_t = wk.tile([128, 2, QLEN], BF16, tag="ElT")
                for ct in range(2):
                    nc.scalar.activation(
                        ElT_t[:, ct, :],
                        St[:, ct, :],
                        AF.Exp,
                        bias=mbi_t[:, b, ct : ct + 1],
                        scale=SCALE,
                    )

                # ============ i-side: F = E^T V (unnormalized i_feat^T... [c,h]) ====
                crud = scores_t[:, 384:400]
                F_sb = wk.tile([128, 2, H], BF16, tag="F_sb")
                for h2 in range(2):
                    F_ps = pp.tile([128, H], F32, tag="F")
                    nc.tensor.matmul(
                        F_ps[:],
                        E_t[:, 128 * h2 : 128 * (h2 + 1)],
                        qn_t[:, bb, :],
                        start=True,
                        stop=True,
                    )
                    # Z column for this c-tile (same weights)
                    nc.tensor.matmul(
                        crud[:, h2 : h2 + 1],
                        E_t[:, 128 * h2 : 128 * (h2 + 1)],
                        ones_bf[0:QLEN, :],
                        start=True,
                        stop=True,
                    )
                    nc.vector.tensor_copy(F_sb[:, h2, :], F_ps[:])
                # F^T [h, c] for the w1 matmul
                FT_sb = wk.tile([128, 4, ILEN], BF16, tag="FT_sb")
                if include_b1:
                    zrow_sb = wk.tile([1, ILEN], BF16, tag="zrow")
                for h2 in range(2):
                    FT_ps = pp.tile([128, 512], F32, tag="FT")
                    for m in (2 * h2, 2 * h2 + 1):
                        nc.tensor.matmul(
                            FT_ps[:, (m % 2) * ILEN : (m % 2 + 1) * ILEN],
                            qn_t[:, bb, 128 * m : 128 * (m + 1)],
                            E_t[:],
                            start=True,
                            stop=True,
                        )
                    if include_b1 and h2 == 0:
                        zrow_ps = pp.tile([1, ILEN], F32, tag="zrow_ps")
                        nc.tensor.matmul(
                            zrow_ps[:], ones_bf[0:QLEN, :], E_t[:], start=True, stop=True
                        )
                        nc.scalar.copy(zrow_sb[:], zrow_ps[:])
                    for m in (2 * h2, 2 * h2 + 1):
                        nc.scalar.copy(
                            FT_sb[:, m, :], FT_ps[:, (m % 2) * ILEN : (m % 2 + 1) * ILEN]
                        )

                # ============ l-side: G = sum_c ElT[c,a] i[c,h]  [a,h] ============
                G_ps = pp.tile([QLEN, H], F32, tag="G")
                for ct in range(2):
                    nc.tensor.matmul(
                        G_ps[:],
                        ElT_t[:, ct, :],
                        inat_t[:, bb * 2 + ct, :],
                        start=(ct == 0),
                        stop=(ct == 1),
                    )
                    nc.tensor.matmul(
                        crud[0:QLEN, 2:3],
                        ElT_t[:, ct, :],
                        ones_bf[:],
                        start=(ct == 0),
                        stop=(ct == 1),
                        skip_group_check=True,
                    )
                G_sb = wk.tile([QLEN, H], BF16, tag="G_sb")
                nc.vector.tensor_copy(G_sb[:], G_ps[:])
                # Zl = col2 + col3 (two single-tile partial sums)
                recipZl_sb = wk.tile([QLEN, 1], F32, tag="recipZl")
                nc.vector.reciprocal(recipZl_sb[:], crud[0:QLEN, 2:3])
                GT_sb = wk.tile([128, 4, QLEN], BF16, tag="GT_sb")
                GT_ps = pp.tile([128, 4, QLEN], F32, tag="GT")
                for m in range(4):
                    for ct in range(2):
                        nc.tensor.matmul(
                            GT_ps[:, m, :],
                            inat_t[:, bb * 2 + ct, 128 * m : 128 * (m + 1)],
                            ElT_t[:, ct, :],
                            start=(ct == 0),
                            stop=(ct == 1),
                        )
                nc.scalar.copy(GT_sb[:], GT_ps[:])
                if include_b1:
                    zlrow_ps = pp.tile([1, QLEN], F32, tag="zlrow_ps")
                    zlrow_sb = wk.tile([1, QLEN], BF16, tag="zlrow")
                    for ct in range(2):
                        nc.tensor.matmul(
                            zlrow_ps[:],
                            ones_bf[:],
                            ElT_t[:, ct, :],
                            start=(ct == 0),
                            stop=(ct == 1),
                        )
                    nc.scalar.copy(zlrow_sb[:], zlrow_ps[:])

                # ============ i-side MLP: h1T = relu(w1^T @ F^T (+ b1 x Z)) ======
                h1T_sb = wk.tile([128, 4, ILEN], BF16, tag="h1T_sb")
                for h2 in range(2):
                    h1T_ps = pp.tile([128, 512], F32, tag="h1")
                    for m in (2 * h2, 2 * h2 + 1):
                        for k in range(4):
                            nc.tensor.matmul(
                                h1T_ps[:, (m % 2) * ILEN : (m % 2 + 1) * ILEN],
                                w1lf_t[:, k, 128 * m : 128 * (m + 1)],
                                FT_sb[:, k, :],
                                start=(k == 0),
                                stop=(k == 3) and not include_b1,
                            )
                        if include_b1:
                            nc.tensor.matmul(
                                h1T_ps[:, (m % 2) * ILEN : (m % 2 + 1) * ILEN],
                                b1lf_t[:, 128 * m : 128 * (m + 1)],
                                zrow_sb[:],
                                start=False,
                                stop=True,
                            )
                    nc.scalar.activation(
                        h1T_sb[:, 2 * h2 : 2 * h2 + 2, :].rearrange("p m c -> p (m c)"),
                        h1T_ps[:],
                        AF.Relu,
                    )
                # s1 columns (unnormalized by Z): s1u[c] = sum_m h1T[m,c] w2[m]
                for j in range(2):
                    for mk in range(4):
                        nc.tensor.matmul(
                            crud[:, 4 + j : 5 + j],
                            h1T_sb[:, mk, 128 * j : 128 * (j + 1)],
                            w2lf_t[:, mk : mk + 1],
                            start=(mk == 0),
                            stop=(mk == 3),
                        )
                # recipZ, e1 = exp(s1u * recipZ + abi)
                recipZ_sb = wk.tile([128, 2], F32, tag="recipZ")
                nc.vector.reciprocal(recipZ_sb[:], crud[:, 0:2])
                e1_sb = wk.tile([128, 2], F32, tag="e1")
                for j in range(2):
                    nc.scalar.activation(
                        e1_sb[:, j : j + 1],
                        crud[:, 4 + j : 5 + j],
                        AF.Exp,
                        bias=abi_t[:, b, j : j + 1],
                        scale=recipZ_sb[:, j : j + 1],
                    )
                # Se, broadcast reciprocal
                for j in range(2):
                    nc.tensor.matmul(
                        crud[0:1, 6:7],
                        e1_sb[:, j : j + 1],
                        ones_f[:],
                        start=(j == 0),
                        stop=(j == 1),
                    )
                rSe_sb = wk.tile([1, 1], F32, tag="rSe")
                nc.vector.reciprocal(rSe_sb[:], crud[0:1, 6:7])
                nc.tensor.matmul(
                    crud[:, 7:8], onesr_f[:], rSe_sb[:], start=True, stop=True
                )
                # att (i_weight output) and pool weights
                for j in range(2):
                    nc.vector.tensor_mul(
                        att_all[:, b, j : j + 1], e1_sb[:, j : j + 1], crud[:, 7:8]
                    )
                wpool_sb = wk.tile([128, 2], BF16, tag="wpool")
                nc.vector.tensor_mul(wpool_sb[:], att_all[:, b, :], recipZ_sb[:])
                # pooledT_i column b
                for m in range(4):
                    for t in range(2):
                        nc.tensor.matmul(
                            pooled_ps[:, m, 0, b : b + 1],
                            F_sb[:, t, 128 * m : 128 * (m + 1)],
                            wpool_sb[:, t : t + 1],
                            start=(t == 0),
                            stop=(t == 1),
                        )

                # ============ l-side MLP ============
                h1Tl_ps = pp.tile([128, 4, QLEN], F32, tag="h1")
                for m in range(4):
                    for k in range(4):
                        nc.tensor.matmul(
                            h1Tl_ps[:, m, :],
                            w1if_t[:, k, 128 * m : 128 * (m + 1)],
                            GT_sb[:, k, :],
                            start=(k == 0),
                            stop=(k == 3) and not include_b1,
                        )
                    if include_b1:
                        nc.tensor.matmul(
                            h1Tl_ps[:, m, :],
                            b1if_t[:, 128 * m : 128 * (m + 1)],
                            zlrow_sb[:],
                            start=False,
                            stop=True,
                        )
                h1Tl_sb = wk.tile([128, 4, QLEN], BF16, tag="h1Tl_sb")
                nc.scalar.activation(
                    h1Tl_sb[:].rearrange("p m c -> p (m c)"),
                    h1Tl_ps[:].rearrange("p m c -> p (m c)"),
                    AF.Relu,
                )
                for mk in range(4):
                    nc.tensor.matmul(
                        crud[0:QLEN, 8:9],
                        h1Tl_sb[:, mk, :],
                        w2if_t[:, mk : mk + 1],
                        start=(mk == 0),
                        stop=(mk == 3),
                    )
                e1l_sb = wk.tile([QLEN, 1], F32, tag="e1l")
                nc.scalar.activation(
                    e1l_sb[:],
                    crud[0:QLEN, 8:9],
                    AF.Exp,
                    bias=abq_t[:, b : b + 1],
                    scale=recipZl_sb[:],
                )
                nc.tensor.matmul(
                    crud[0:1, 9:10], e1l_sb[:], ones_f[0:QLEN, :], start=True, stop=True
                )
                rSel_sb = wk.tile([1, 1], F32, tag="rSel")
                nc.vector.reciprocal(rSel_sb[:], crud[0:1, 9:10])
                nc.tensor.matmul(
                    crud[0:QLEN, 10:11],
                    onesr_f[:, 0:QLEN],
                    rSel_sb[:],
                    start=True,
                    stop=True,
                )
                wpl_sb = wk.tile([QLEN, 1], BF16, tag="wpl")
                attl_sb = wk.tile([QLEN, 1], F32, tag="attl")
                nc.vector.tensor_mul(attl_sb[:], e1l_sb[:], crud[0:QLEN, 10:11])
                nc.vector.tensor_mul(wpl_sb[:], attl_sb[:], recipZl_sb[:])
                for m in range(4):
                    nc.tensor.matmul(
                        pooled_ps[:, m, 1, b : b + 1],
                        G_sb[:, 128 * m : 128 * (m + 1)],
                        wpl_sb[:],
                        start=True,
                        stop=True,
                    )

        # ================= phase 2: batched tail =================
        nc.vector.tensor_copy(
            pooled_sb[:].rearrange("p a b c -> p (a b c)"),
            pooled_ps[:].rearrange("p a b c -> p (a b c)"),
        )
        flatT_ps = pp.tile([128, 8, C], F32, tag="G")
        for j in range(8):
            for s, wm_t in enumerate((wmlf_t, wmif_t)):
                for k in range(4):
                    nc.tensor.matmul(
                        flatT_ps[:, j, :],
                        wm_t[:, k, 128 * j : 128 * (j + 1)],
                        pooled_sb[:, k, s, :],
                        start=(s == 0 and k == 0),
                        stop=(s == 1 and k == 3),
                    )
        nc.scalar.copy(
            flatT_sb[:].rearrange("p a b -> p (a b)"),
            flatT_ps[:].rearrange("p a b -> p (a b)"),
        )
        outT_ps = pp.tile([128, 4, C], F32, tag="GT")
        for j in range(4):
            for k in range(8):
                nc.tensor.matmul(
                    outT_ps[:, j, :],
                    fh_t[:, k, 128 * j : 128 * (j + 1)],
                    flatT_sb[:, k, :],
                    start=(k == 0),
                    stop=(k == 7),
                )
            nc.scalar.activation(
                outT_sb[:, :, j],
                outT_ps[:, j, :],
                AF.Identity,
                bias=fhb_t[:, j : j + 1],
                scale=1.0,
            )
        nc.sync.dma_start(
            out=outp.rearrange("b (j p) -> p (b j)", p=128), in_=outT_sb[:]
        )
        nc.sync.dma_start(
            out=iwp.rearrange("b (t p) -> p (b t)", p=128), in_=att_all[:]
        )
    nc.finalize()
    return nc


_CACHE = {}


def _get_program(C, NB, include_b1):
    key = (C, NB, include_b1)
    if key not in _CACHE:
        _CACHE[key] = build(C, NB, include_b1)
    return _CACHE[key]


def make_in_maps(inputs, C=B // NCORES):
    bf = ml_dtypes.bfloat16
    i_bf = np.ascontiguousarray(inputs["i_batch"]).astype(bf)
    q_bf = np.ascontiguousarray(inputs["q_batch"]).astype(bf)
    qm = np.asarray(inputs["q_mask"]).reshape(-1, QLEN).astype(bool)
    im = np.asarray(inputs["i_mask"]).reshape(-1, ILEN).astype(bool)
    mbq = np.where(qm, np.float32(-1e9), np.float32(0.0)).astype(np.float32)
    mbi = np.where(im, np.float32(-1e9), np.float32(0.0)).astype(np.float32)
    abi = (mbi + np.float32(inputs["lf_b2"][0])).astype(np.float32)
    abq = (mbq + np.float32(inputs["if_b2"][0])).astype(np.float32)
    fhb_eff = (
        np.asarray(inputs["fh_b"])
        + (np.asarray(inputs["lf_bm"]) + np.asarray(inputs["if_bm"]))
        @ np.asarray(inputs["fh_w"])
    ).astype(np.float32)
    consts = dict(
        w1lf=np.asarray(inputs["lf_w1"]).astype(bf),
        w1if=np.asarray(inputs["if_w1"]).astype(bf),
        w2lf=np.asarray(inputs["lf_w2"])[:, 0].astype(bf),
        w2if=np.asarray(inputs["if_w2"])[:, 0].astype(bf),
        wmlf=np.asarray(inputs["lf_wm"]).astype(bf),
        wmif=np.asarray(inputs["if_wm"]).astype(bf),
        fhw=np.asarray(inputs["fh_w"]).astype(bf),
        fhb=fhb_eff,
        b1lf=np.asarray(inputs["lf_b1"]).astype(bf),
        b1if=np.asarray(inputs["if_b1"]).astype(bf),
    )
    iT_full = np.ascontiguousarray(i_bf.transpose(0, 2, 1))
    qT_full = np.ascontiguousarray(q_bf.transpose(0, 2, 1))
    nb = i_bf.shape[0] // C
    in_maps = []
    for c in range(nb):
        s = slice(c * C, (c + 1) * C)
        in_maps.append(
            dict(
                i_bf=np.ascontiguousarray(i_bf[s]),
                q_bf=np.ascontiguousarray(q_bf[s]),
                iT_d=iT_full[s],
                qT_d=qT_full[s],
                mbq=np.ascontiguousarray(mbq[s]),
                mbi=np.ascontiguousarray(mbi[s]),
                abi=np.ascontiguousarray(abi[s]),
                abq=np.ascontiguousarray(abq[s]),
                **consts,
            )
        )
    include_b1 = bool(np.any(inputs["lf_b1"]) or np.any(inputs["if_b1"]))
    return in_maps, include_b1


TRACE = False
LAST_RESULTS = None


def kernel(**inputs):
    global LAST_RESULTS
    C = B // NCORES
    in_maps, include_b1 = make_in_maps(inputs, C)
    nc = _get_program(C, 4, include_b1)
    res = run_bass_kernel_spmd(nc, in_maps, list(range(NCORES)), trace=TRACE)
    LAST_RESULTS = res
    out = np.concatenate([res.results[c]["outp"] for c in range(NCORES)], axis=0)
    iw = np.concatenate([res.results[c]["iwp"] for c in range(NCORES)], axis=0)
    return out.astype(np.float32), iw.astype(np.float32)


# revision 15
# speedup vs baseline: 1.1078x; 1.1078x over previous
"""Trainium2 Bass kernel for a BiAttention module (data-parallel over batch).

Computes, per batch item b:
  i_feat = attn(q=i_b, kv=q_b, mask=q_mask)        [256, 512]
  i_flat, i_weight = att_flat(i_feat, i_mask, lf_*)
  l_feat = attn(q=q_b, kv=i_b, mask=i_mask)        [64, 512]
  l_flat, _ = att_flat(l_feat, q_mask, if_*)
  out = (l_flat + i_flat) @ fh_w + fh_b            [512]
Returns (out [B,512], i_weight [B,256]).

Strategy: 8-way data parallel over batch (32 items/core), weights replicated.
Single score matrix S = q_b @ i_b^T serves both attentions (S and S^T computed
on PE from DMA-transposed bf16 operand layouts). Softmax uses exp without max
subtraction (scores are O(1); masked lanes get -1e9 bias -> exp underflows to
exactly 0). Softmax-Z normalizations are deferred through the relu MLP via
positive homogeneity and applied as per-partition scalars.
"""

import numpy as np
import ml_dtypes
from contextlib import ExitStack

import concourse.bass as bass
import concourse.bacc as bacc
import concourse.mybir as mybir
import concourse.tile as tile
from concourse.bass_utils import run_bass_kernel_spmd

B, ILEN, QLEN, H = 256, 256, 64, 512
MID, FLAT, OUT = 512, 1024, 512
NCORES = 8
F32 = mybir.dt.float32
BF16 = mybir.dt.bfloat16
SCALE = float(1.0 / np.sqrt(H))
AF = mybir.ActivationFunctionType


def build(C=B // NCORES, NB=4, include_b1=False):
    nc = bacc.Bacc()

    # ---- DRAM parameters (inputs) ----
    i_bf = nc.declare_dram_parameter("i_bf", [C, ILEN, H], BF16, isOutput=False)
    q_bf = nc.declare_dram_parameter("q_bf", [C, QLEN, H], BF16, isOutput=False)
    iT_d = nc.declare_dram_parameter("iT_d", [C, H, ILEN], BF16, isOutput=False)
    qT_d = nc.declare_dram_parameter("qT_d", [C, H, QLEN], BF16, isOutput=False)
    mbq = nc.declare_dram_parameter("mbq", [C, QLEN], F32, isOutput=False)
    mbi = nc.declare_dram_parameter("mbi", [C, ILEN], F32, isOutput=False)
    abi = nc.declare_dram_parameter("abi", [C, ILEN], F32, isOutput=False)
    abq = nc.declare_dram_parameter("abq", [C, QLEN], F32, isOutput=False)
    w1lf = nc.declare_dram_parameter("w1lf", [H, MID], BF16, isOutput=False)
    w1if = nc.declare_dram_parameter("w1if", [H, MID], BF16, isOutput=False)
    w2lf = nc.declare_dram_parameter("w2lf", [MID], BF16, isOutput=False)
    w2if = nc.declare_dram_parameter("w2if", [MID], BF16, isOutput=False)
    wmlf = nc.declare_dram_parameter("wmlf", [H, FLAT], BF16, isOutput=False)
    wmif = nc.declare_dram_parameter("wmif", [H, FLAT], BF16, isOutput=False)
    fhw = nc.declare_dram_parameter("fhw", [FLAT, OUT], BF16, isOutput=False)
    fhb = nc.declare_dram_parameter("fhb", [OUT], F32, isOutput=False)
    b1lf = nc.declare_dram_parameter("b1lf", [MID], BF16, isOutput=False)
    b1if = nc.declare_dram_parameter("b1if", [MID], BF16, isOutput=False)
    # ---- outputs ----
    outp = nc.declare_dram_parameter("outp", [C, OUT], F32, isOutput=True)
    iwp = nc.declare_dram_parameter("iwp", [C, ILEN], F32, isOutput=True)

    with ExitStack() as ctx:
        tc = ctx.enter_context(tile.TileContext(nc))
        const = ctx.enter_context(tc.tile_pool(name="const", bufs=1))
        iop = ctx.enter_context(tc.tile_pool(name="iop", bufs=3))
        wk = ctx.enter_context(tc.tile_pool(name="wk", bufs=3))
        pp = ctx.enter_context(tc.tile_pool(name="pp", bufs=1, space="PSUM"))

        # ---- constants / weights resident in SBUF ----
        w1lf_t = const.tile([128, 4, MID], BF16, tag="w1lf")
        nc.sync.dma_start(out=w1lf_t[:], in_=w1lf.rearrange("(k p) m -> p k m", p=128))
        w1if_t = const.tile([128, 4, MID], BF16, tag="w1if")
        nc.sync.dma_start(out=w1if_t[:], in_=w1if.rearrange("(k p) m -> p k m", p=128))
        w2lf_t = const.tile([128, 4], BF16, tag="w2lf")
        nc.sync.dma_start(out=w2lf_t[:], in_=w2lf.rearrange("(k p) -> p k", p=128))
        w2if_t = const.tile([128, 4], BF16, tag="w2if")
        nc.sync.dma_start(out=w2if_t[:], in_=w2if.rearrange("(k p) -> p k", p=128))
        wmlf_t = const.tile([128, 4, FLAT], BF16, tag="wmlf")
        nc.sync.dma_start(out=wmlf_t[:], in_=wmlf.rearrange("(k p) m -> p k m", p=128))
        wmif_t = const.tile([128, 4, FLAT], BF16, tag="wmif")
        nc.sync.dma_start(out=wmif_t[:], in_=wmif.rearrange("(k p) m -> p k m", p=128))
        fh_t = const.tile([128, 8, OUT], BF16, tag="fh")
        nc.sync.dma_start(out=fh_t[:], in_=fhw.rearrange("(k p) m -> p k m", p=128))
        fhb_t = const.tile([128, 4], F32, tag="fhb")
        nc.sync.dma_start(out=fhb_t[:], in_=fhb.rearrange("(j p) -> p j", p=128))

        mbq_t = const.tile([QLEN, C], F32, tag="mbq")
        nc.sync.dma_start(out=mbq_t[:], in_=mbq.rearrange("b a -> a b"))
        mbi_t = const.tile([128, C, 2], F32, tag="mbi")
        nc.sync.dma_start(out=mbi_t[:], in_=mbi.rearrange("b (t p) -> p (b t)", p=128))
        abi_t = const.tile([128, C, 2], F32, tag="abi")
        nc.sync.dma_start(out=abi_t[:], in_=abi.rearrange("b (t p) -> p (b t)", p=128))
        abq_t = const.tile([QLEN, C], F32, tag="abq")
        nc.sync.dma_start(out=abq_t[:], in_=abq.rearrange("b a -> a b"))

        ones_bf = const.tile([128, 1], BF16, tag="ones_bf")
        nc.vector.memset(ones_bf[:], 1.0)
        ones_f = const.tile([128, 1], F32, tag="ones_f")
        nc.vector.memset(ones_f[:], 1.0)
        onesr_f = const.tile([1, 128], F32, tag="onesr_f")
        nc.vector.memset(onesr_f[:], 1.0)

        if include_b1:
            b1lf_t = const.tile([1, MID], BF16, tag="b1lf")
            nc.sync.dma_start(out=b1lf_t[:], in_=b1lf.rearrange("(o m) -> o m", o=1))
            b1if_t = const.tile([1, MID], BF16, tag="b1if")
            nc.sync.dma_start(out=b1if_t[:], in_=b1if.rearrange("(o m) -> o m", o=1))

        # persistent accumulators / output staging
        att_all = const.tile([128, C, 2], F32, tag="att_all")
        pooled_sb = const.tile([128, 4, 2, C], BF16, tag="pooled_sb")
        flatT_sb = const.tile([128, 8, C], BF16, tag="flatT_sb")
        outT_sb = const.tile([128, C, 4], F32, tag="outT_sb")
        pooled_ps = pp.tile([128, 4, 2, C], F32, tag="pooled")

        n_groups = C // NB
        for g in range(n_groups):
            bsl = slice(g * NB, (g + 1) * NB)
            # --- transposed layouts (h on partitions), pre-transposed on host ---
            iT_t = iop.tile([128, NB, 4, ILEN], BF16, tag="iT")
            nc.sync.dma_start(
                out=iT_t[:], in_=iT_d[bsl].rearrange("b (k p) c -> p (b k) c", p=128)
            )
            qT_t = iop.tile([128, NB, 4, QLEN], BF16, tag="qT")
            nc.sync.dma_start(
                out=qT_t[:], in_=qT_d[bsl].rearrange("b (k p) c -> p (b k) c", p=128)
            )
            # --- natural loads (seq on partitions) ---
            inat_t = iop.tile([128, NB * 2, H], BF16, tag="inat")
            nc.sync.dma_start(
                out=inat_t[:], in_=i_bf[bsl].rearrange("b (t p) h -> p (b t) h", p=128)
            )
            qn_t = iop.tile([QLEN, NB, H], BF16, tag="qn")
            nc.sync.dma_start(out=qn_t[:], in_=q_bf[bsl].rearrange("b r h -> r b h"))

            for bb in range(NB):
                b = g * NB + bb
                # ============ scores: S [64,256] and S^T [256,64] ============
                scores_t = pp.tile([128, 512], F32, tag="scores", bufs=2)
                St = scores_t[:, 0:128].rearrange("p (t a) -> p t a", a=64)
                S = scores_t[0:QLEN, 128 : 128 + ILEN]
                for k in range(4):
                    nc.tensor.matmul(
                        S,
                        qT_t[:, bb, k, :],
                        iT_t[:, bb, k, :],
                        start=(k == 0),
                        stop=(k == 3),
                    )
                for ct in range(2):
                    for k in range(4):
                        nc.tensor.matmul(
                            St[:, ct, :],
                            iT_t[:, bb, k, 128 * ct : 128 * (ct + 1)],
                            qT_t[:, bb, k, :],
                            start=(k == 0),
                            stop=(k == 3),
                        )
                # ============ masked exps ============
                E_t = wk.tile([QLEN, ILEN], BF16, tag="E")
                nc.scalar.activation(
                    E_t[:], S, AF.Exp, bias=mbq_t[:, b : b + 1], scale=SCALE
                )
                ElT_t = wk.tile([128, 2, QLEN], BF16, tag="ElT")
                for ct in range(2):
                    nc.scalar.activation(
                        ElT_t[:, ct, :],
                        St[:, ct, :],
                        AF.Exp,
                        bias=mbi_t[:, b, ct : ct + 1],
                        scale=SCALE,
                    )

                # ============ i-side: F = E^T V (unnormalized i_feat^T... [c,h]) ====
                crud = scores_t[:, 384:400]
                F_sb = wk.tile([128, 2, H], BF16, tag="F_sb")
                for h2 in range(2):
                    F_ps = pp.tile([128, H], F32, tag="F")
                    nc.tensor.matmul(
                        F_ps[:],
                        E_t[:, 128 * h2 : 128 * (h2 + 1)],
                        qn_t[:, bb, :],
                        start=True,
                        stop=True,
                    )
                    # Z column for this c-tile (same weights)
                    nc.tensor.matmul(
                        crud[:, h2 : h2 + 1],
                        E_t[:, 128 * h2 : 128 * (h2 + 1)],
                        ones_bf[0:QLEN, :],
                        start=True,
                        stop=True,
                    )
                    nc.vector.tensor_copy(F_sb[:, h2, :], F_ps[:])
                # F^T [h, c] for the w1 matmul
                FT_sb = wk.tile([128, 4, ILEN], BF16, tag="FT_sb")
                if include_b1:
                    zrow_sb = wk.tile([1, ILEN], BF16, tag="zrow")
                for h2 in range(2):
                    FT_ps = pp.tile([128, 512], F32, tag="FT")
                    for m in (2 * h2, 2 * h2 + 1):
                        nc.tensor.matmul(
                            FT_ps[:, (m % 2) * ILEN : (m % 2 + 1) * ILEN],
                            qn_t[:, bb, 128 * m : 128 * (m + 1)],
                            E_t[:],
                            start=True,
                            stop=True,
                        )
                    if include_b1 and h2 == 0:
                        zrow_ps = pp.tile([1, ILEN], F32, tag="zrow_ps")
                        nc.tensor.matmul(
                            zrow_ps[:], ones_bf[0:QLEN, :], E_t[:], start=True, stop=True
                        )
                        nc.scalar.copy(zrow_sb[:], zrow_ps[:])
                    for m in (2 * h2, 2 * h2 + 1):
                        nc.scalar.copy(
                            FT_sb[:, m, :], FT_ps[:, (m % 2) * ILEN : (m % 2 + 1) * ILEN]
                        )

                # ============ l-side: G = sum_c ElT[c,a] i[c,h]  [a,h] ============
                G_ps = pp.tile([QLEN, H], F32, tag="G")
                for ct in range(2):
                    nc.tensor.matmul(
                        G_ps[:],
                        ElT_t[:, ct, :],
                        inat_t[:, bb * 2 + ct, :],
                        start=(ct == 0),
                        stop=(ct == 1),
                    )
                    nc.tensor.matmul(
                        crud[0:QLEN, 2:3],
                        ElT_t[:, ct, :],
                        ones_bf[:],
                        start=(ct == 0),
                        stop=(ct == 1),
                        skip_group_check=True,
                    )
                G_sb = wk.tile([QLEN, H], BF16, tag="G_sb")
                nc.vector.tensor_copy(G_sb[:], G_ps[:])
                # Zl = col2 + col3 (two single-tile partial sums)
                recipZl_sb = wk.tile([QLEN, 1], F32, tag="recipZl")
                nc.vector.reciprocal(recipZl_sb[:], crud[0:QLEN, 2:3])
                GT_sb = wk.tile([128, 4, QLEN], BF16, tag="GT_sb")
                GT_ps = pp.tile([128, 4, QLEN], F32, tag="GT")
                for m in range(4):
                    for ct in range(2):
                        nc.tensor.matmul(
                            GT_ps[:, m, :],
                            inat_t[:, bb * 2 + ct, 128 * m : 128 * (m + 1)],
                            ElT_t[:, ct, :],
                            start=(ct == 0),
                            stop=(ct == 1),
                        )
                nc.scalar.copy(GT_sb[:], GT_ps[:])
                if include_b1:
                    zlrow_ps = pp.tile([1, QLEN], F32, tag="zlrow_ps")
                    zlrow_sb = wk.tile([1, QLEN], BF16, tag="zlrow")
                    for ct in range(2):
                        nc.tensor.matmul(
                            zlrow_ps[:],
                            ones_bf[:],
                            ElT_t[:, ct, :],
                            start=(ct == 0),
                            stop=(ct == 1),
                        )
                    nc.scalar.copy(zlrow_sb[:], zlrow_ps[:])

                # ============ i-side MLP: h1T = relu(w1^T @ F^T (+ b1 x Z)) ======
                h1T_sb = wk.tile([128, 4, ILEN], BF16, tag="h1T_sb")
                for h2 in range(2):
                    h1T_ps = pp.tile([128, 512], F32, tag="h1")
                    for m in (2 * h2, 2 * h2 + 1):
                        for k in range(4):
                            nc.tensor.matmul(
                                h1T_ps[:, (m % 2) * ILEN : (m % 2 + 1) * ILEN],
                                w1lf_t[:, k, 128 * m : 128 * (m + 1)],
                                FT_sb[:, k, :],
                                start=(k == 0),
                                stop=(k == 3) and not include_b1,
                            )
                        if include_b1:
                            nc.tensor.matmul(
                                h1T_ps[:, (m % 2) * ILEN : (m % 2 + 1) * ILEN],
                                b1lf_t[:, 128 * m : 128 * (m + 1)],
                                zrow_sb[:],
                                start=False,
                                stop=True,
                            )
                    nc.scalar.activation(
                        h1T_sb[:, 2 * h2 : 2 * h2 + 2, :].rearrange("p m c -> p (m c)"),
                        h1T_ps[:],
                        AF.Relu,
                    )
                # s1 columns (unnormalized by Z): s1u[c] = sum_m h1T[m,c] w2[m]
                for j in range(2):
                    for mk in range(4):
                        nc.tensor.matmul(
                            crud[:, 4 + j : 5 + j],
                            h1T_sb[:, mk, 128 * j : 128 * (j + 1)],
                            w2lf_t[:, mk : mk + 1],
                            start=(mk == 0),
                            stop=(mk == 3),
                        )
                # recipZ, e1 = exp(s1u * recipZ + abi)
                recipZ_sb = wk.tile([128, 2], F32, tag="recipZ")
                nc.vector.reciprocal(recipZ_sb[:], crud[:, 0:2])
                e1_sb = wk.tile([128, 2], F32, tag="e1")
                for j in range(2):
                    nc.scalar.activation(
                        e1_sb[:, j : j + 1],
                        crud[:, 4 + j : 5 + j],
                        AF.Exp,
                        bias=abi_t[:, b, j : j + 1],
                        scale=recipZ_sb[:, j : j + 1],
                    )
                # Se, broadcast reciprocal
                for j in range(2):
                    nc.tensor.matmul(
                        crud[0:1, 6:7],
                        e1_sb[:, j : j + 1],
                        ones_f[:],
                        start=(j == 0),
                        stop=(j == 1),
                    )
                rSe_sb = wk.tile([1, 1], F32, tag="rSe")
                nc.vector.reciprocal(rSe_sb[:], crud[0:1, 6:7])
                nc.tensor.matmul(
                    crud[:, 7:8], onesr_f[:], rSe_sb[:], start=True, stop=True
                )
                # att (i_weight output) and pool weights
                for j in range(2):
                    nc.vector.tensor_mul(
                        att_all[:, b, j : j + 1], e1_sb[:, j : j + 1], crud[:, 7:8]
                    )
                wpool_sb = wk.tile([128, 2], BF16, tag="wpool")
                nc.vector.tensor_mul(wpool_sb[:], att_all[:, b, :], recipZ_sb[:])
                # pooledT_i column b
                for m in range(4):
                    for t in range(2):
                        nc.tensor.matmul(
                            pooled_ps[:, m, 0, b : b + 1],
                            F_sb[:, t, 128 * m : 128 * (m + 1)],
                            wpool_sb[:, t : t + 1],
                            start=(t == 0),
                            stop=(t == 1),
                        )

                # ============ l-side MLP ============
                h1Tl_ps = pp.tile([128, 4, QLEN], F32, tag="h1")
                for m in range(4):
                    for k in range(4):
                        nc.tensor.matmul(
                            h1Tl_ps[:, m, :],
                            w1if_t[:, k, 128 * m : 128 * (m + 1)],
                            GT_sb[:, k, :],
                            start=(k == 0),
                            stop=(k == 3) and not include_b1,
                        )
                    if include_b1:
                        nc.tensor.matmul(
                            h1Tl_ps[:, m, :],
                            b1if_t[:, 128 * m : 128 * (m + 1)],
                            zlrow_sb[:],
                            start=False,
                            stop=True,
                        )
                h1Tl_sb = wk.tile([128, 4, QLEN], BF16, tag="h1Tl_sb")
                nc.scalar.activation(
                    h1Tl_sb[:].rearrange("p m c -> p (m c)"),
                    h1Tl_ps[:].rearrange("p m c -> p (m c)"),
                    AF.Relu,
                )
                for mk in range(4):
                    nc.tensor.matmul(
                        crud[0:QLEN, 8:9],
                        h1Tl_sb[:, mk, :],
                        w2if_t[:, mk : mk + 1],
                        start=(mk == 0),
                        stop=(mk == 3),
                    )
                e1l_sb = wk.tile([QLEN, 1], F32, tag="e1l")
                nc.scalar.activation(
                    e1l_sb[:],
                    crud[0:QLEN, 8:9],
                    AF.Exp,
                    bias=abq_t[:, b : b + 1],
                    scale=recipZl_sb[:],
                )
                nc.tensor.matmul(
                    crud[0:1, 9:10], e1l_sb[:], ones_f[0:QLEN, :], start=True, stop=True
                )
                rSel_sb = wk.tile([1, 1], F32, tag="rSel")
                nc.vector.reciprocal(rSel_sb[:], crud[0:1, 9:10])
                nc.tensor.matmul(
                    crud[0:QLEN, 10:11],
                    onesr_f[:, 0:QLEN],
                    rSel_sb[:],
                    start=True,
                    stop=True,
                )
                wpl_sb = wk.tile([QLEN, 1], BF16, tag="wpl")
                attl_sb = wk.tile([QLEN, 1], F32, tag="attl")
                nc.vector.tensor_mul(attl_sb[:], e1l_sb[:], crud[0:QLEN, 10:11])
                nc.vector.tensor_mul(wpl_sb[:], attl_sb[:], recipZl_sb[:])
                for m in range(4):
                    nc.tensor.matmul(
                        pooled_ps[:, m, 1, b : b + 1],
                        G_sb[:, 128 * m : 128 * (m + 1)],
                        wpl_sb[:],
                        start=True,
                        stop=True,
                    )

        # ================= phase 2: batched tail =================
        nc.vector.tensor_copy(
            pooled_sb[:].rearrange("p a b c -> p (a b c)"),
            pooled_ps[:].rearrange("p a b c -> p (a b c)"),
        )
        flatT_ps = pp.tile([128, 8, C], F32, tag="G")
        for j in range(8):
            for s, wm_t in enumerate((wmlf_t, wmif_t)):
                for k in range(4):
                    nc.tensor.matmul(
                        flatT_ps[:, j, :],
                        wm_t[:, k, 128 * j : 128 * (j + 1)],
                        pooled_sb[:, k, s, :],
                        start=(s == 0 and k == 0),
                        stop=(s == 1 and k == 3),
                    )
        nc.scalar.copy(
            flatT_sb[:].rearrange("p a b -> p (a b)"),
            flatT_ps[:].rearrange("p a b -> p (a b)"),
        )
        outT_ps = pp.tile([128, 4, C], F32, tag="GT")
        for j in range(4):
            for k in range(8):
                nc.tensor.matmul(
                    outT_ps[:, j, :],
                    fh_t[:, k, 128 * j : 128 * (j + 1)],
                    flatT_sb[:, k, :],
                    start=(k == 0),
                    stop=(k == 7),
                )
            nc.scalar.activation(
                outT_sb[:, :, j],
                outT_ps[:, j, :],
                AF.Identity,
                bias=fhb_t[:, j : j + 1],
                scale=1.0,
            )
        nc.sync.dma_start(
            out=outp.rearrange("b (j p) -> p (b j)", p=128), in_=outT_sb[:]
        )
        nc.sync.dma_start(
            out=iwp.rearrange("b (t p) -> p (b t)", p=128), in_=att_all[:]
        )
    nc.finalize()
    return nc


_CACHE = {}


def _get_program(C, NB, include_b1):
    key = (C, NB, include_b1)
    if key not in _CACHE:
        _CACHE[key] = build(C, NB, include_b1)
    return _CACHE[key]


def make_in_maps(inputs, C=B // NCORES):
    bf = ml_dtypes.bfloat16
    i_bf = np.ascontiguousarray(inputs["i_batch"]).astype(bf)
    q_bf = np.ascontiguousarray(inputs["q_batch"]).astype(bf)
    qm = np.asarray(inputs["q_mask"]).reshape(-1, QLEN).astype(bool)
    im = np.asarray(inputs["i_mask"]).reshape(-1, ILEN).astype(bool)
    mbq = np.where(qm, np.float32(-1e9), np.float32(0.0)).astype(np.float32)
    mbi = np.where(im, np.float32(-1e9), np.float32(0.0)).astype(np.float32)
    abi = (mbi + np.float32(inputs["lf_b2"][0])).astype(np.float32)
    abq = (mbq + np.float32(inputs["if_b2"][0])).astype(np.float32)
    fhb_eff = (
        np.asarray(inputs["fh_b"])
        + (np.asarray(inputs["lf_bm"]) + np.asarray(inputs["if_bm"]))
        @ np.asarray(inputs["fh_w"])
    ).astype(np.float32)
    consts = dict(
        w1lf=np.asarray(inputs["lf_w1"]).astype(bf),
        w1if=np.asarray(inputs["if_w1"]).astype(bf),
        w2lf=np.asarray(inputs["lf_w2"])[:, 0].astype(bf),
        w2if=np.asarray(inputs["if_w2"])[:, 0].astype(bf),
        wmlf=np.asarray(inputs["lf_wm"]).astype(bf),
        wmif=np.asarray(inputs["if_wm"]).astype(bf),
        fhw=np.asarray(inputs["fh_w"]).astype(bf),
        fhb=fhb_eff,
        b1lf=np.asarray(inputs["lf_b1"]).astype(bf),
        b1if=np.asarray(inputs["if_b1"]).astype(bf),
    )
    iT_full = np.ascontiguousarray(i_bf.transpose(0, 2, 1))
    qT_full = np.ascontiguousarray(q_bf.transpose(0, 2, 1))
    nb = i_bf.shape[0] // C
    in_maps = []
    for c in range(nb):
        s = slice(c * C, (c + 1) * C)
        in_maps.append(
            dict(
                i_bf=np.ascontiguousarray(i_bf[s]),
                q_bf=np.ascontiguousarray(q_bf[s]),
                iT_d=iT_full[s],
                qT_d=qT_full[s],
                mbq=np.ascontiguousarray(mbq[s]),
                mbi=np.ascontiguousarray(mbi[s]),
                abi=np.ascontiguousarray(abi[s]),
                abq=np.ascontiguousarray(abq[s]),
                **consts,
            )
        )
    include_b1 = bool(np.any(inputs["lf_b1"]) or np.any(inputs["if_b1"]))
    return in_maps, include_b1


TRACE = False
LAST_RESULTS = None


def kernel(**inputs):
    global LAST_RESULTS
    C = B // NCORES
    in_maps, include_b1 = make_in_maps(inputs, C)
    nc = _get_program(C, 4, include_b1)
    res = run_bass_kernel_spmd(nc, in_maps, list(range(NCORES)), trace=TRACE)
    LAST_RESULTS = res
    out = np.concatenate([res.results[c]["outp"] for c in range(NCORES)], axis=0)
    iw = np.concatenate([res.results[c]["iwp"] for c in range(NCORES)], axis=0)
    return out.astype(np.float32), iw.astype(np.float32)


# revision 16
# speedup vs baseline: 1.1382x; 1.0274x over previous
"""Trainium2 Bass kernel for a BiAttention module (data-parallel over batch).

Computes, per batch item b:
  i_feat = attn(q=i_b, kv=q_b, mask=q_mask)        [256, 512]
  i_flat, i_weight = att_flat(i_feat, i_mask, lf_*)
  l_feat = attn(q=q_b, kv=i_b, mask=i_mask)        [64, 512]
  l_flat, _ = att_flat(l_feat, q_mask, if_*)
  out = (l_flat + i_flat) @ fh_w + fh_b            [512]
Returns (out [B,512], i_weight [B,256]).

Strategy: 8-way data parallel over batch (32 items/core), weights replicated.
Single score matrix S = q_b @ i_b^T serves both attentions (S and S^T computed
on PE from DMA-transposed bf16 operand layouts). Softmax uses exp without max
subtraction (scores are O(1); masked lanes get -1e9 bias -> exp underflows to
exactly 0). Softmax-Z normalizations are deferred through the relu MLP via
positive homogeneity and applied as per-partition scalars.
"""

import numpy as np
import ml_dtypes
from contextlib import ExitStack

import concourse.bass as bass
import concourse.bacc as bacc
import concourse.mybir as mybir
import concourse.tile as tile
from concourse.bass_utils import run_bass_kernel_spmd

B, ILEN, QLEN, H = 256, 256, 64, 512
MID, FLAT, OUT = 512, 1024, 512
NCORES = 8
F32 = mybir.dt.float32
BF16 = mybir.dt.bfloat16
SCALE = float(1.0 / np.sqrt(H))
AF = mybir.ActivationFunctionType


def build(C=B // NCORES, NB=4, include_b1=False):
    nc = bacc.Bacc()

    # ---- DRAM parameters (inputs) ----
    i_bf = nc.declare_dram_parameter("i_bf", [C, ILEN, H], BF16, isOutput=False)
    q_bf = nc.declare_dram_parameter("q_bf", [C, QLEN, H], BF16, isOutput=False)
    iT_d = nc.declare_dram_parameter("iT_d", [C, H, ILEN], BF16, isOutput=False)
    qT_d = nc.declare_dram_parameter("qT_d", [C, H, QLEN], BF16, isOutput=False)
    mbq = nc.declare_dram_parameter("mbq", [QLEN, C], F32, isOutput=False)
    mbi = nc.declare_dram_parameter("mbi", [128, C, 2], F32, isOutput=False)
    abi = nc.declare_dram_parameter("abi", [128, C, 2], F32, isOutput=False)
    abq = nc.declare_dram_parameter("abq", [QLEN, C], F32, isOutput=False)
    w1lf = nc.declare_dram_parameter("w1lf", [H, MID], BF16, isOutput=False)
    w1if = nc.declare_dram_parameter("w1if", [H, MID], BF16, isOutput=False)
    w2lf = nc.declare_dram_parameter("w2lf", [MID], BF16, isOutput=False)
    w2if = nc.declare_dram_parameter("w2if", [MID], BF16, isOutput=False)
    wmlf = nc.declare_dram_parameter("wmlf", [H, FLAT], BF16, isOutput=False)
    wmif = nc.declare_dram_parameter("wmif", [H, FLAT], BF16, isOutput=False)
    fhw = nc.declare_dram_parameter("fhw", [FLAT, OUT], BF16, isOutput=False)
    fhb = nc.declare_dram_parameter("fhb", [OUT], F32, isOutput=False)
    b1lf = nc.declare_dram_parameter("b1lf", [MID], BF16, isOutput=False)
    b1if = nc.declare_dram_parameter("b1if", [MID], BF16, isOutput=False)
    # ---- outputs ----
    outp = nc.declare_dram_parameter("outp", [C, OUT], F32, isOutput=True)
    iwp = nc.declare_dram_parameter("iwp", [C, ILEN], F32, isOutput=True)

    with ExitStack() as ctx:
        tc = ctx.enter_context(tile.TileContext(nc))
        const = ctx.enter_context(tc.tile_pool(name="const", bufs=1))
        iop = ctx.enter_context(tc.tile_pool(name="iop", bufs=3))
        wk = ctx.enter_context(tc.tile_pool(name="wk", bufs=3))
        pp = ctx.enter_context(tc.tile_pool(name="pp", bufs=1, space="PSUM"))

        def load_group(g):
            bsl = slice(g * NB, (g + 1) * NB)
            iT_t = iop.tile([128, NB, 4, ILEN], BF16, tag="iT", name="iT_t")
            nc.sync.dma_start(
                out=iT_t[:], in_=iT_d[bsl].rearrange("b (k p) c -> p (b k) c", p=128)
            )
            qT_t = iop.tile([128, NB, 4, QLEN], BF16, tag="qT", name="qT_t")
            nc.sync.dma_start(
                out=qT_t[:], in_=qT_d[bsl].rearrange("b (k p) c -> p (b k) c", p=128)
            )
            inat_t = iop.tile([128, NB * 2, H], BF16, tag="inat", name="inat_t")
            nc.sync.dma_start(
                out=inat_t[:], in_=i_bf[bsl].rearrange("b (t p) h -> p (b t) h", p=128)
            )
            qn_t = iop.tile([QLEN, NB, H], BF16, tag="qn", name="qn_t")
            nc.sync.dma_start(out=qn_t[:], in_=q_bf[bsl].rearrange("b r h -> r b h"))
            return iT_t, qT_t, inat_t, qn_t

        group0 = load_group(0)

        # ---- constants / weights resident in SBUF ----
        w1lf_t = const.tile([128, 4, MID], BF16, tag="w1lf")
        nc.sync.dma_start(out=w1lf_t[:], in_=w1lf.rearrange("(k p) m -> p k m", p=128))
        w1if_t = const.tile([128, 4, MID], BF16, tag="w1if")
        nc.sync.dma_start(out=w1if_t[:], in_=w1if.rearrange("(k p) m -> p k m", p=128))
        w2lf_t = const.tile([128, 4], BF16, tag="w2lf")
        nc.sync.dma_start(out=w2lf_t[:], in_=w2lf.rearrange("(k p) -> p k", p=128))
        w2if_t = const.tile([128, 4], BF16, tag="w2if")
        nc.sync.dma_start(out=w2if_t[:], in_=w2if.rearrange("(k p) -> p k", p=128))
        wmlf_t = const.tile([128, 4, FLAT], BF16, tag="wmlf")
        nc.sync.dma_start(out=wmlf_t[:], in_=wmlf.rearrange("(k p) m -> p k m", p=128))
        wmif_t = const.tile([128, 4, FLAT], BF16, tag="wmif")
        nc.sync.dma_start(out=wmif_t[:], in_=wmif.rearrange("(k p) m -> p k m", p=128))
        fh_t = const.tile([128, 8, OUT], BF16, tag="fh")
        nc.sync.dma_start(out=fh_t[:], in_=fhw.rearrange("(k p) m -> p k m", p=128))
        fhb_t = const.tile([128, 4], F32, tag="fhb")
        nc.sync.dma_start(out=fhb_t[:], in_=fhb.rearrange("(j p) -> p j", p=128))

        mbq_t = const.tile([QLEN, C], F32, tag="mbq")
        nc.sync.dma_start(out=mbq_t[:], in_=mbq[:])
        mbi_t = const.tile([128, C, 2], F32, tag="mbi")
        nc.sync.dma_start(out=mbi_t[:], in_=mbi[:])
        abi_t = const.tile([128, C, 2], F32, tag="abi")
        nc.sync.dma_start(out=abi_t[:], in_=abi[:])
        abq_t = const.tile([QLEN, C], F32, tag="abq")
        nc.sync.dma_start(out=abq_t[:], in_=abq[:])

        ones_bf = const.tile([128, 1], BF16, tag="ones_bf")
        nc.vector.memset(ones_bf[:], 1.0)
        ones_f = const.tile([128, 1], F32, tag="ones_f")
        nc.vector.memset(ones_f[:], 1.0)
        onesr_f = const.tile([1, 128], F32, tag="onesr_f")
        nc.vector.memset(onesr_f[:], 1.0)

        if include_b1:
            b1lf_t = const.tile([1, MID], BF16, tag="b1lf")
            nc.sync.dma_start(out=b1lf_t[:], in_=b1lf.rearrange("(o m) -> o m", o=1))
            b1if_t = const.tile([1, MID], BF16, tag="b1if")
            nc.sync.dma_start(out=b1if_t[:], in_=b1if.rearrange("(o m) -> o m", o=1))

        # persistent accumulators / output staging
        att_all = const.tile([128, C, 2], F32, tag="att_all")
        pooled_sb = const.tile([128, 4, 2, C], BF16, tag="pooled_sb")
        flatT_sb = const.tile([128, 8, C], BF16, tag="flatT_sb")
        outT_sb = const.tile([128, C, 4], F32, tag="outT_sb")
        pooled_ps = pp.tile([128, 4, 2, C], F32, tag="pooled")

        n_groups = C // NB
        for g in range(n_groups):
            iT_t, qT_t, inat_t, qn_t = group0 if g == 0 else load_group(g)

            for bb in range(NB):
                b = g * NB + bb
                # ============ scores: S [64,256] and S^T [256,64] ============
                scores_t = pp.tile([128, 512], F32, tag="scores", bufs=2)
                St = scores_t[:, 0:128].rearrange("p (t a) -> p t a", a=64)
                S = scores_t[0:QLEN, 128 : 128 + ILEN]
                for k in range(4):
                    nc.tensor.matmul(
                        S,
                        qT_t[:, bb, k, :],
                        iT_t[:, bb, k, :],
                        start=(k == 0),
                        stop=(k == 3),
                    )
                for ct in range(2):
                    for k in range(4):
                        nc.tensor.matmul(
                            St[:, ct, :],
                            iT_t[:, bb, k, 128 * ct : 128 * (ct + 1)],
                            qT_t[:, bb, k, :],
                            start=(k == 0),
                            stop=(k == 3),
                        )
                # ============ masked exps ============
                E_t = wk.tile([QLEN, ILEN], BF16, tag="E")
                nc.scalar.activation(
                    E_t[:], S, AF.Exp, bias=mbq_t[:, b : b + 1], scale=SCALE
                )
                ElT_t = wk.tile([128, 2, QLEN], BF16, tag="ElT")
                for ct in range(2):
                    nc.scalar.activation(
                        ElT_t[:, ct, :],
                        St[:, ct, :],
                        AF.Exp,
                        bias=mbi_t[:, b, ct : ct + 1],
                        scale=SCALE,
                    )

                # ============ i-side: F = E^T V (unnormalized i_feat^T... [c,h]) ====
                crud = scores_t[:, 384:400]
                F_sb = wk.tile([128, 2, H], BF16, tag="F_sb")
                for h2 in range(2):
                    F_ps = pp.tile([128, H], F32, tag="F")
                    nc.tensor.matmul(
                        F_ps[:],
                        E_t[:, 128 * h2 : 128 * (h2 + 1)],
                        qn_t[:, bb, :],
                        start=True,
                        stop=True,
                    )
                    # Z column for this c-tile (same weights)
                    nc.tensor.matmul(
                        crud[:, h2 : h2 + 1],
                        E_t[:, 128 * h2 : 128 * (h2 + 1)],
                        ones_bf[0:QLEN, :],
                        start=True,
                        stop=True,
                    )
                    nc.vector.tensor_copy(F_sb[:, h2, :], F_ps[:])
                # F^T [h, c] for the w1 matmul
                FT_sb = wk.tile([128, 4, ILEN], BF16, tag="FT_sb")
                if include_b1:
                    zrow_sb = wk.tile([1, ILEN], BF16, tag="zrow")
                for h2 in range(2):
                    FT_ps = pp.tile([128, 512], F32, tag="FT")
                    for m in (2 * h2, 2 * h2 + 1):
                        nc.tensor.matmul(
                            FT_ps[:, (m % 2) * ILEN : (m % 2 + 1) * ILEN],
                            qn_t[:, bb, 128 * m : 128 * (m + 1)],
                            E_t[:],
                            start=True,
                            stop=True,
                        )
                    if include_b1 and h2 == 0:
                        zrow_ps = pp.tile([1, ILEN], F32, tag="zrow_ps")
                        nc.tensor.matmul(
                            zrow_ps[:], ones_bf[0:QLEN, :], E_t[:], start=True, stop=True
                        )
                        nc.scalar.copy(zrow_sb[:], zrow_ps[:])
                    for m in (2 * h2, 2 * h2 + 1):
                        nc.scalar.copy(
                            FT_sb[:, m, :], FT_ps[:, (m % 2) * ILEN : (m % 2 + 1) * ILEN]
                        )

                # ============ l-side: G = sum_c ElT[c,a] i[c,h]  [a,h] ============
                G_ps = pp.tile([QLEN, H], F32, tag="G")
                for ct in range(2):
                    nc.tensor.matmul(
                        G_ps[:],
                        ElT_t[:, ct, :],
                        inat_t[:, bb * 2 + ct, :],
                        start=(ct == 0),
                        stop=(ct == 1),
                    )
                    nc.tensor.matmul(
                        crud[0:QLEN, 2:3],
                        ElT_t[:, ct, :],
                        ones_bf[:],
                        start=(ct == 0),
                        stop=(ct == 1),
                        skip_group_check=True,
                    )
                G_sb = wk.tile([QLEN, H], BF16, tag="G_sb")
                nc.vector.tensor_copy(G_sb[:], G_ps[:])
                # Zl = col2 + col3 (two single-tile partial sums)
                recipZl_sb = wk.tile([QLEN, 1], F32, tag="recipZl")
                nc.vector.reciprocal(recipZl_sb[:], crud[0:QLEN, 2:3])
                GT_sb = wk.tile([128, 4, QLEN], BF16, tag="GT_sb")
                GT_ps = pp.tile([128, 4, QLEN], F32, tag="GT")
                for m in range(4):
                    for ct in range(2):
                        nc.tensor.matmul(
                            GT_ps[:, m, :],
                            inat_t[:, bb * 2 + ct, 128 * m : 128 * (m + 1)],
                            ElT_t[:, ct, :],
                            start=(ct == 0),
                            stop=(ct == 1),
                        )
                nc.scalar.copy(GT_sb[:], GT_ps[:])
                if include_b1:
                    zlrow_ps = pp.tile([1, QLEN], F32, tag="zlrow_ps")
                    zlrow_sb = wk.tile([1, QLEN], BF16, tag="zlrow")
                    for ct in range(2):
                        nc.tensor.matmul(
                            zlrow_ps[:],
                            ones_bf[:],
                            ElT_t[:, ct, :],
                            start=(ct == 0),
                            stop=(ct == 1),
                        )
                    nc.scalar.copy(zlrow_sb[:], zlrow_ps[:])

                # ============ i-side MLP: h1T = relu(w1^T @ F^T (+ b1 x Z)) ======
                h1T_sb = wk.tile([128, 4, ILEN], BF16, tag="h1T_sb")
                for h2 in range(2):
                    h1T_ps = pp.tile([128, 512], F32, tag="h1")
                    for m in (2 * h2, 2 * h2 + 1):
                        for k in range(4):
                            nc.tensor.matmul(
                                h1T_ps[:, (m % 2) * ILEN : (m % 2 + 1) * ILEN],
                                w1lf_t[:, k, 128 * m : 128 * (m + 1)],
                                FT_sb[:, k, :],
                                start=(k == 0),
                                stop=(k == 3) and not include_b1,
                            )
                        if include_b1:
                            nc.tensor.matmul(
                                h1T_ps[:, (m % 2) * ILEN : (m % 2 + 1) * ILEN],
                                b1lf_t[:, 128 * m : 128 * (m + 1)],
                                zrow_sb[:],
                                start=False,
                                stop=True,
                            )
                    nc.scalar.activation(
                        h1T_sb[:, 2 * h2 : 2 * h2 + 2, :].rearrange("p m c -> p (m c)"),
                        h1T_ps[:],
                        AF.Relu,
                    )
                # s1 columns (unnormalized by Z): s1u[c] = sum_m h1T[m,c] w2[m]
                for j in range(2):
                    for mk in range(4):
                        nc.tensor.matmul(
                            crud[:, 4 + j : 5 + j],
                            h1T_sb[:, mk, 128 * j : 128 * (j + 1)],
                            w2lf_t[:, mk : mk + 1],
                            start=(mk == 0),
                            stop=(mk == 3),
                        )
                # recipZ, e1 = exp(s1u * recipZ + abi)
                recipZ_sb = wk.tile([128, 2], F32, tag="recipZ")
                nc.vector.reciprocal(recipZ_sb[:], crud[:, 0:2])
                e1_sb = wk.tile([128, 2], F32, tag="e1")
                for j in range(2):
                    nc.scalar.activation(
                        e1_sb[:, j : j + 1],
                        crud[:, 4 + j : 5 + j],
                        AF.Exp,
                        bias=abi_t[:, b, j : j + 1],
                        scale=recipZ_sb[:, j : j + 1],
                    )
                # Se, broadcast reciprocal
                for j in range(2):
                    nc.tensor.matmul(
                        crud[0:1, 6:7],
                        e1_sb[:, j : j + 1],
                        ones_f[:],
                        start=(j == 0),
                        stop=(j == 1),
                    )
                rSe_sb = wk.tile([1, 1], F32, tag="rSe")
                nc.vector.reciprocal(rSe_sb[:], crud[0:1, 6:7])
                nc.tensor.matmul(
                    crud[:, 7:8], onesr_f[:], rSe_sb[:], start=True, stop=True
                )
                # att (i_weight output) and pool weights
                for j in range(2):
                    nc.vector.tensor_mul(
                        att_all[:, b, j : j + 1], e1_sb[:, j : j + 1], crud[:, 7:8]
                    )
                wpool_sb = wk.tile([128, 2], BF16, tag="wpool")
                nc.vector.tensor_mul(wpool_sb[:], att_all[:, b, :], recipZ_sb[:])
                # pooledT_i column b
                for m in range(4):
                    for t in range(2):
                        nc.tensor.matmul(
                            pooled_ps[:, m, 0, b : b + 1],
                            F_sb[:, t, 128 * m : 128 * (m + 1)],
                            wpool_sb[:, t : t + 1],
                            start=(t == 0),
                            stop=(t == 1),
                        )

                # ============ l-side MLP ============
                h1Tl_ps = pp.tile([128, 4, QLEN], F32, tag="h1")
                for m in range(4):
                    for k in range(4):
                        nc.tensor.matmul(
                            h1Tl_ps[:, m, :],
                            w1if_t[:, k, 128 * m : 128 * (m + 1)],
                            GT_sb[:, k, :],
                            start=(k == 0),
                            stop=(k == 3) and not include_b1,
                        )
                    if include_b1:
                        nc.tensor.matmul(
                            h1Tl_ps[:, m, :],
                            b1if_t[:, 128 * m : 128 * (m + 1)],
                            zlrow_sb[:],
                            start=False,
                            stop=True,
                        )
                h1Tl_sb = wk.tile([128, 4, QLEN], BF16, tag="h1Tl_sb")
                nc.scalar.activation(
                    h1Tl_sb[:].rearrange("p m c -> p (m c)"),
                    h1Tl_ps[:].rearrange("p m c -> p (m c)"),
                    AF.Relu,
                )
                for mk in range(4):
                    nc.tensor.matmul(
                        crud[0:QLEN, 8:9],
                        h1Tl_sb[:, mk, :],
                        w2if_t[:, mk : mk + 1],
                        start=(mk == 0),
                        stop=(mk == 3),
                    )
                e1l_sb = wk.tile([QLEN, 1], F32, tag="e1l")
                nc.scalar.activation(
                    e1l_sb[:],
                    crud[0:QLEN, 8:9],
                    AF.Exp,
                    bias=abq_t[:, b : b + 1],
                    scale=recipZl_sb[:],
                )
                nc.tensor.matmul(
                    crud[0:1, 9:10], e1l_sb[:], ones_f[0:QLEN, :], start=True, stop=True
                )
                rSel_sb = wk.tile([1, 1], F32, tag="rSel")
                nc.vector.reciprocal(rSel_sb[:], crud[0:1, 9:10])
                nc.tensor.matmul(
                    crud[0:QLEN, 10:11],
                    onesr_f[:, 0:QLEN],
                    rSel_sb[:],
                    start=True,
                    stop=True,
                )
                wpl_sb = wk.tile([QLEN, 1], BF16, tag="wpl")
                attl_sb = wk.tile([QLEN, 1], F32, tag="attl")
                nc.vector.tensor_mul(attl_sb[:], e1l_sb[:], crud[0:QLEN, 10:11])
                nc.vector.tensor_mul(wpl_sb[:], attl_sb[:], recipZl_sb[:])
                for m in range(4):
                    nc.tensor.matmul(
                        pooled_ps[:, m, 1, b : b + 1],
                        G_sb[:, 128 * m : 128 * (m + 1)],
                        wpl_sb[:],
                        start=True,
                        stop=True,
                    )

        # ================= phase 2: batched tail =================
        nc.vector.tensor_copy(
            pooled_sb[:].rearrange("p a b c -> p (a b c)"),
            pooled_ps[:].rearrange("p a b c -> p (a b c)"),
        )
        flatT_ps = pp.tile([128, 8, C], F32, tag="G")
        for j in range(8):
            for s, wm_t in enumerate((wmlf_t, wmif_t)):
                for k in range(4):
                    nc.tensor.matmul(
                        flatT_ps[:, j, :],
                        wm_t[:, k, 128 * j : 128 * (j + 1)],
                        pooled_sb[:, k, s, :],
                        start=(s == 0 and k == 0),
                        stop=(s == 1 and k == 3),
                    )
        nc.scalar.copy(
            flatT_sb[:].rearrange("p a b -> p (a b)"),
            flatT_ps[:].rearrange("p a b -> p (a b)"),
        )
        outT_ps = pp.tile([128, 4, C], F32, tag="GT")
        for j in range(4):
            for k in range(8):
                nc.tensor.matmul(
                    outT_ps[:, j, :],
                    fh_t[:, k, 128 * j : 128 * (j + 1)],
                    flatT_sb[:, k, :],
                    start=(k == 0),
                    stop=(k == 7),
                )
            nc.scalar.activation(
                outT_sb[:, :, j],
                outT_ps[:, j, :],
                AF.Identity,
                bias=fhb_t[:, j : j + 1],
                scale=1.0,
            )
        nc.sync.dma_start(
            out=outp.rearrange("b (j p) -> p (b j)", p=128), in_=outT_sb[:]
        )
        nc.sync.dma_start(
            out=iwp.rearrange("b (t p) -> p (b t)", p=128), in_=att_all[:]
        )
    nc.finalize()
    return nc


_CACHE = {}


def _get_program(C, NB, include_b1):
    key = (C, NB, include_b1)
    if key not in _CACHE:
        _CACHE[key] = build(C, NB, include_b1)
    return _CACHE[key]


def make_in_maps(inputs, C=B // NCORES):
    bf = ml_dtypes.bfloat16
    i_bf = np.ascontiguousarray(inputs["i_batch"]).astype(bf)
    q_bf = np.ascontiguousarray(inputs["q_batch"]).astype(bf)
    qm = np.asarray(inputs["q_mask"]).reshape(-1, QLEN).astype(bool)
    im = np.asarray(inputs["i_mask"]).reshape(-1, ILEN).astype(bool)
    mbq = np.where(qm, np.float32(-1e9), np.float32(0.0)).astype(np.float32)
    mbi = np.where(im, np.float32(-1e9), np.float32(0.0)).astype(np.float32)
    abi = (mbi + np.float32(inputs["lf_b2"][0])).astype(np.float32)
    abq = (mbq + np.float32(inputs["if_b2"][0])).astype(np.float32)
    fhb_eff = (
        np.asarray(inputs["fh_b"])
        + (np.asarray(inputs["lf_bm"]) + np.asarray(inputs["if_bm"]))
        @ np.asarray(inputs["fh_w"])
    ).astype(np.float32)
    consts = dict(
        w1lf=np.asarray(inputs["lf_w1"]).astype(bf),
        w1if=np.asarray(inputs["if_w1"]).astype(bf),
        w2lf=np.asarray(inputs["lf_w2"])[:, 0].astype(bf),
        w2if=np.asarray(inputs["if_w2"])[:, 0].astype(bf),
        wmlf=np.asarray(inputs["lf_wm"]).astype(bf),
        wmif=np.asarray(inputs["if_wm"]).astype(bf),
        fhw=np.asarray(inputs["fh_w"]).astype(bf),
        fhb=fhb_eff,
        b1lf=np.asarray(inputs["lf_b1"]).astype(bf),
        b1if=np.asarray(inputs["if_b1"]).astype(bf),
    )
    iT_full = np.ascontiguousarray(i_bf.transpose(0, 2, 1))
    qT_full = np.ascontiguousarray(q_bf.transpose(0, 2, 1))
    nb = i_bf.shape[0] // C
    in_maps = []
    for c in range(nb):
        s = slice(c * C, (c + 1) * C)
        in_maps.append(
            dict(
                i_bf=np.ascontiguousarray(i_bf[s]),
                q_bf=np.ascontiguousarray(q_bf[s]),
                iT_d=iT_full[s],
                qT_d=qT_full[s],
                mbq=np.ascontiguousarray(mbq[s].T),
                mbi=np.ascontiguousarray(mbi[s].reshape(C, 2, 128).transpose(2, 0, 1)),
                abi=np.ascontiguousarray(abi[s].reshape(C, 2, 128).transpose(2, 0, 1)),
                abq=np.ascontiguousarray(abq[s].T),
                **consts,
            )
        )
    include_b1 = bool(np.any(inputs["lf_b1"]) or np.any(inputs["if_b1"]))
    return in_maps, include_b1


TRACE = False
LAST_RESULTS = None


def kernel(**inputs):
    global LAST_RESULTS
    C = B // NCORES
    in_maps, include_b1 = make_in_maps(inputs, C)
    nc = _get_program(C, 4, include_b1)
    res = run_bass_kernel_spmd(nc, in_maps, list(range(NCORES)), trace=TRACE)
    LAST_RESULTS = res
    out = np.concatenate([res.results[c]["outp"] for c in range(NCORES)], axis=0)
    iw = np.concatenate([res.results[c]["iwp"] for c in range(NCORES)], axis=0)
    return out.astype(np.float32), iw.astype(np.float32)
